# revision 4
# baseline (speedup 1.0000x reference)
"""Trainium2 Bass kernel for a char-LSTM (nn_CharsLstm) — v2: x-path off PE.

Reference computation (B=4096 words, T=30 chars, D=512 emb, H=1024 hidden,
V=128 chars):
    xe = emb[x]                        # [B, T, D]
    scan over t: gates = xt @ W_ih.T + b_ih + h @ W_hh.T + b_hh
                 i, f, g, o = split(gates, 4)
                 c = sig(f)*c + sig(i)*tanh(g); h = sig(o)*tanh(c)
    return h                           # [B, H]

Strategy (delta vs the v1 fp8-DoubleRow baseline):
  - Data parallel: batch 4096 -> 8 cores x 512 words. No collectives.
  - v1 injected the x-path (embedding+input projection folded to
    Wc[4H, V]) as a 5th DoubleRow matmul pair per output tile (one-hot
    moving operand). The PE runs at its DoubleRow cadence limit, so that
    5th pair is 20% of the kernel. v2 removes it: the x contribution
    xw[b, t] = Wc[:, x[b,t]] is gathered ON HOST into a per-step fp16
    tensor ([128, KC, 4, 512] per step, 4 MB/core/step), streamed to SBUF
    a step ahead (double-buffered), and added into the gate pre-activations
    by the DVE (the only non-PE engine with PSUM access; GPSIMD's PSUM
    port is physically absent and DMA has no PSUM route).
  - Per chunk j (128 gate rows x 4 gates), per half ([i,f] / [o,g]):
    4 fp8 DoubleRow matmul pairs (K=1024 h-path) accumulate in a 2-bank
    PSUM half-tile; DVE adds xw and writes the sum to an SBUF f32 staging
    tile (freeing the PSUM banks before the activation, which keeps the
    PE 2 chunks ahead); ACT applies the one sigmoid per half
    (tanh(g) = 2*sigmoid(2g) - 1 recovered by a DVE dual-op as in v1).
  - Engine balance per step (spec model, per core): PE 128 MMs ~23us,
    ACT 16 sigmoids + paired tanh ~23us, DVE 16 adds + t_g ~21us,
    GPSIMD (SBUF-only: m, u, c-update, h-muls) ~20us.
  - tanh(c) batched over chunk pairs (0,1),(2,3),(4,5); chunks 6,7 stay
    single so the step-boundary serial chain (last MM -> add -> sig ->
    c-update -> tanh -> h, whose h chunks 6/7 gate the next step's a3
    pairs) stays short. Front-load of a0-a2 for chunks 0/1 covers it.
  - Weights x16 (g rows x32) for fp8 range as in v1; ACT descales via the
    free scale operand; xw carries the same scaling (fp16 has the range).
"""

import numpy as np
import ml_dtypes

import concourse.bacc as bacc
import concourse.mybir as mybir
import concourse.tile as tile
from concourse.bass_utils import run_bass_kernel_spmd

B, T, D, H, V = 4096, 30, 512, 1024, 128
NCORES = 8
N = B // NCORES          # batch per core (matmul moving free dim)
KC = H // 128            # 8 h-chunks of 128 rows
NPAIR = KC // 2          # 4 DoubleRow h-pairs (x-path is off the PE)
F32 = mybir.dt.float32
F16 = mybir.dt.float16
BF16 = mybir.dt.bfloat16
FP8 = mybir.dt.float8e4
FP8NP = ml_dtypes.float8_e4m3
SIG = mybir.ActivationFunctionType.Sigmoid
TANH = mybir.ActivationFunctionType.Tanh
DR = mybir.MatmulPerfMode.DoubleRow
GSCALE = 16.0            # weights carry x16; ACT descales gates by 1/16
XW_STEP = KC * 4 * N     # fp16 elems per partition per step (16384)

_cached = {}


def build_kernel(n_steps=T, repeat=1):
    nc = bacc.Bacc("TRN2", target_bir_lowering=False)

    # Host-prepared layouts (gate rows permuted to [i, f, o, g] order):
    #  whh  [128, NPAIR*2*4096] fp8:
    #    whh[p, a, r, m] = fp8(16*W_hh[m, (2a+r)*128 + p])   (g rows x32)
    #  xw   [128, n_steps*KC*4*N] fp16:
    #    xw[p, t, j, q, b] = 16*Wc[q*H + j*128 + p, x[b, t]] (g rows x32)
    #    where Wc = W_ih @ emb.T + (b_ih + b_hh)[:, None]
    #  h0t  [128, KC*N]  fp8 : h0t[p, k*N+b] = h0[b, k*128+p]
    #  c0t  [128, KC*N]  f32 : same layout
    #  out  [128, KC*N]  f32 : same layout (host inverts)
    whh_d = nc.dram_tensor("whh", [128, NPAIR * 2 * 4096], FP8,
                           kind="ExternalInput")
    xw_d = nc.dram_tensor("xw", [128, n_steps * XW_STEP], F16,
                          kind="ExternalInput")
    h0_d = nc.dram_tensor("h0t", [128, KC * N], FP8, kind="ExternalInput")
    c0_d = nc.dram_tensor("c0t", [128, KC * N], F32, kind="ExternalInput")
    out_d = nc.dram_tensor("out", [128, KC * N], F32, kind="ExternalOutput")

    with tile.TileContext(nc) as tc:
        with (
            tc.tile_pool(name="weights", bufs=1) as wpool,
            tc.tile_pool(name="state", bufs=2) as spool,
            tc.tile_pool(name="xws", bufs=2) as xpool,
            tc.tile_pool(name="tmps", bufs=4) as tpool,
            # two half-chunk PSUM pools (2 banks each): the [i,f] half
            # drains while the [o,g] matmuls still run, and PSUM recycles
            # at half-chunk granularity
            tc.tile_pool(name="psum_if", bufs=2, space="PSUM") as pool_if,
            tc.tile_pool(name="psum_og", bufs=2, space="PSUM") as pool_og,
        ):
            # DMA emission order = consumption order.
            wcomb = wpool.tile([128, NPAIR, 2, 4096], FP8, tag="wcomb")
            nc.sync.dma_start(out=wcomb[:, 0, :, :], in_=whh_d[:, 0:8192])
            ht = spool.tile([128, KC, N], FP8, tag="ht")
            nc.sync.dma_start(out=ht[:, :, :], in_=h0_d[:, :])
            for a in range(1, NPAIR):
                nc.sync.dma_start(out=wcomb[:, a, :, :],
                                  in_=whh_d[:, a * 8192:(a + 1) * 8192])
            # c tiles: pairs (0,1),(2,3),(4,5) share a tile so tanh can
            # batch both chunks in one ACT instruction; 6 and 7 stay
            # separate (short tail chain at the step boundary)
            ctp = {}
            for j0 in (0, 2, 4):
                tile_c = wpool.tile([128, 2, 512], F32, tag=f"ct{j0}",
                                    name=f"ct{j0}")
                nc.sync.dma_start(out=tile_c,
                                  in_=c0_d[:, j0 * N:(j0 + 2) * N])
                ctp[j0] = ctp[j0 + 1] = tile_c
            for j in (6, 7):
                tile_c = wpool.tile([128, 512], F32, tag=f"ct{j}",
                                    name=f"ct{j}")
                nc.sync.dma_start(out=tile_c, in_=c0_d[:, j * N:(j + 1) * N])
                ctp[j] = tile_c

            def ct(j):
                t_ = ctp[j]
                return t_[:, j % 2, :] if j < 6 else t_

            ht_fin = wpool.tile([128, KC, N], F32, tag="ht_fin")

            # stage2(j) = tanh(c_new) + h-mul, deferred ~2 chunks behind
            # stage1 (queue carries across steps) so a tanh(c) never sits
            # at the head of the in-order ACT queue waiting on its c-update.
            # Chunks pair up for a batched tanh; 6/7 flush as singles at the
            # step boundary (AFTER the front-load matmuls that only read h
            # chunks 0-5, BEFORE the a3 pairs that read h chunks 6/7 —
            # emission order defines the dependency graph).
            pend = []

            def emit_stage2_pair(args):
                j0, s_o0, s_o1, h_dst, dma_j = args
                t_c = tpool.tile([128, 2, 512], BF16, tag="t_c")
                nc.scalar.activation(out=t_c, in_=ctp[j0], func=TANH)
                nc.gpsimd.tensor_mul(h_dst[:, j0, :], s_o0, t_c[:, 0, :])
                nc.gpsimd.tensor_mul(h_dst[:, j0 + 1, :], s_o1, t_c[:, 1, :])
                if dma_j is not None:
                    nc.sync.dma_start(
                        out=out_d[:, j0 * N:(j0 + 2) * N],
                        in_=h_dst[:, j0:j0 + 2, :])

            def emit_stage2_single(args):
                j, s_o, h_dst, dma_j = args
                t_c = tpool.tile([128, 512], BF16, tag="t_c1")
                nc.scalar.activation(out=t_c, in_=ctp[j], func=TANH)
                nc.vector.tensor_mul(h_dst[:, j, :], s_o, t_c)
                if dma_j is not None:
                    nc.sync.dma_start(out=out_d[:, j * N:(j + 1) * N],
                                      in_=h_dst[:, j, :])

            total = n_steps * repeat
            for s in range(total):
                t = s % n_steps
                last = s == total - 1
                ht_next = None if last else spool.tile([128, KC, N], FP8,
                                                       tag="ht")
                h_dst_t = ht_fin if last else ht_next

                # stream this step's x contribution; with bufs=2 the DMA
                # runs during step s-1 (buffer s%2 frees when step s-2's
                # adds finish)
                xw = xpool.tile([128, KC, 4, N], F16, tag="xw",
                                name=f"xw_{s}")
                nc.sync.dma_start(out=xw,
                                  in_=xw_d[:, t * XW_STEP:(t + 1) * XW_STEP])

                def emit_mms(pt, j, qs, a_list):
                    # h-pairs in k order: last-produced h chunks needed last
                    for a in a_list:
                        for qi, q in enumerate(qs):
                            m0 = q * H + j * 128
                            nc.tensor.matmul(
                                pt[:, qi, :],
                                wcomb[:, a, :, m0:m0 + 128],
                                ht[:, 2 * a:2 * a + 2, :],
                                start=(a == 0), stop=(a == NPAIR - 1),
                                perf_mode=DR,
                            )

                def emit_half(pt, j, qs, xlo, s4, slo):
                    # DVE adds the host-gathered xw into the PSUM gates and
                    # stages the sum in SBUF f32 (frees the PSUM banks
                    # before the sigmoid); ACT descales by 1/16 in the
                    # activation's free scale operand
                    spre = tpool.tile([128, 2, 512], F32, tag="spre")
                    nc.vector.tensor_add(spre, pt, xw[:, j, xlo:xlo + 2, :])
                    nc.scalar.activation(out=s4[:, slo:slo + 2, :], in_=spre,
                                         func=SIG, scale=1.0 / GSCALE)

                def emit_chunk(j, front=None):
                    if front is None:
                        ptif = pool_if.tile([128, 2, 512], F32, tag="pif",
                                            name=f"pif_{s}_{j}")
                        ptog = pool_og.tile([128, 2, 512], F32, tag="pog",
                                            name=f"pog_{s}_{j}")
                        emit_mms(ptif, j, (0, 1), range(NPAIR))
                    else:
                        ptif, ptog = front
                        emit_mms(ptif, j, (0, 1), [NPAIR - 1])
                    s4 = tpool.tile([128, 4, 512], BF16, tag="s4")
                    emit_half(ptif, j, (0, 1), 0, s4, 0)
                    m = tpool.tile([128, 512], F32, tag="m")
                    nc.gpsimd.tensor_mul(m, ct(j), s4[:, 1, :])
                    if front is None:
                        emit_mms(ptog, j, (2, 3), range(NPAIR))
                    else:
                        emit_mms(ptog, j, (2, 3), [NPAIR - 1])
                    emit_half(ptog, j, (2, 3), 2, s4, 2)
                    # tanh(g) = 2*sigmoid(2g) - 1 (g weights carry the x2)
                    t_g = tpool.tile([128, 512], BF16, tag="t_g")
                    nc.vector.tensor_scalar(t_g, s4[:, 3, :], 2.0, -1.0,
                                            mybir.AluOpType.mult,
                                            mybir.AluOpType.add)
                    u = tpool.tile([128, 512], BF16, tag="u")
                    nc.gpsimd.tensor_mul(u, s4[:, 0, :], t_g)
                    nc.gpsimd.tensor_add(ct(j), m, u)
                    push_stage2(j, s4[:, 2, :])

                def push_stage2(j, s_o, last=last, h_dst_t=h_dst_t):
                    dma_j = j if last else None
                    pend.append((j, s_o, h_dst_t, dma_j))
                    # pairs flush two chunks deferred: (0,1) after chunk 2,
                    # (2,3) after 4, (4,5) after 6; 6/7 flush as singles at
                    # the next step boundary
                    if len(pend) >= 3 and pend[0][0] % 2 == 0 and pend[0][0] < 6:
                        j0, s_o0, hd, dj0 = pend.pop(0)
                        _, s_o1, _, dj1 = pend.pop(0)
                        emit_stage2_pair((j0, s_o0, s_o1, hd,
                                          j0 if dj0 is not None else None))

                def flush_pend():
                    while pend:
                        if (len(pend) >= 2 and pend[0][0] % 2 == 0
                                and pend[0][0] < 6
                                and pend[1][0] == pend[0][0] + 1):
                            j0, s_o0, hd, dj0 = pend.pop(0)
                            _, s_o1, _, dj1 = pend.pop(0)
                            emit_stage2_pair((j0, s_o0, s_o1, hd,
                                              j0 if dj0 is not None else None))
                        else:
                            emit_stage2_single(pend.pop(0))

                # Step boundary: open the four half-tiles of chunks 0/1 and
                # front-load pairs a0-a2 (24 MMs of cover, reading only h
                # chunks 0-5) before the pending stage2 singles for h chunks
                # 6/7 flush; the a3 pairs (readers of h 6/7) come after.
                fr = {}
                for j in (0, 1):
                    fr[j] = (pool_if.tile([128, 2, 512], F32, tag="pif",
                                          name=f"pif_{s}_{j}"),
                             pool_og.tile([128, 2, 512], F32, tag="pog",
                                          name=f"pog_{s}_{j}"))
                    emit_mms(fr[j][0], j, (0, 1), range(NPAIR - 1))
                    emit_mms(fr[j][1], j, (2, 3), range(NPAIR - 1))
                flush_pend()
                emit_chunk(0, front=fr[0])
                emit_chunk(1, front=fr[1])
                for j in range(2, KC):
                    emit_chunk(j)
                ht = ht_next
            flush_pend()

    nc.compile()
    return nc


def _prep_core_inputs(x, wcomb, h0, c0, core, n_steps=T):
    whh, xtab = wcomb
    sl = slice(core * N, (core + 1) * N)
    x_c = np.asarray(x[sl]).astype(np.int64)     # [N, T]
    # int16 view: np.take on float16 falls off numpy's fast path (~17x)
    xv = xtab.view(np.int16)
    xw_all = np.empty((128, n_steps, KC, 4, N), dtype=np.int16)
    for t in range(n_steps):
        # xtab[p, j, q, v] -> gather chars of step t: [128, KC, 4, N]
        xw_all[:, t] = np.take(xv, x_c[:, t], axis=3)
    xw_all = xw_all.reshape(128, n_steps * XW_STEP).view(np.float16)
    h0t = np.ascontiguousarray(
        h0[sl].reshape(N, KC, 128).transpose(2, 1, 0).reshape(128, KC * N)
    ).astype(FP8NP)
    c0t = np.ascontiguousarray(
        c0[sl].reshape(N, KC, 128).transpose(2, 1, 0).reshape(128, KC * N)
    ).astype(np.float32)
    return {"whh": whh, "xw": xw_all, "h0t": h0t, "c0t": c0t}


def _prep_weights(emb, W_ih, W_hh, b_ih, b_hh):
    # gate reorder [i, f, o, g]
    perm = np.concatenate([np.arange(0, H), np.arange(H, 2 * H),
                           np.arange(3 * H, 4 * H), np.arange(2 * H, 3 * H)])
    # g rows (block 3 after reorder) carry an extra x2: the single sigmoid
    # yields sigmoid(2g) there and tanh(g) = 2*sig(2g) - 1
    gate_scale = np.repeat([1.0, 1.0, 1.0, 2.0], H)[:, None] * GSCALE
    whh_s = gate_scale * W_hh[perm]                      # [4H, H] x16 (g x32)

    # whh[p, a, r, m] = whh_s[m, (2a+r)*128 + p]
    whh = np.zeros((128, NPAIR, 2, 4 * H), dtype=np.float32)
    for a in range(NPAIR):
        for r in range(2):
            k = 2 * a + r
            whh[:, a, r, :] = whh_s[:, k * 128:(k + 1) * 128].T
    whh = np.ascontiguousarray(
        whh.reshape(128, NPAIR * 2 * 4096)).astype(FP8NP)

    # x-path table for the host gather: xtab[p, j, q, v]
    wc = W_ih @ emb.T + (b_ih + b_hh)[:, None]           # [4H, V]
    wc = gate_scale * wc[perm]                           # x16 (g x32)
    xtab = np.ascontiguousarray(
        wc.reshape(4, KC, 128, V).transpose(2, 1, 0, 3)).astype(np.float16)
    return whh, xtab


def kernel(x, emb, W_ih, W_hh, b_ih, b_hh, h0, c0, n_steps=T):
    x = np.asarray(x)
    emb = np.asarray(emb, dtype=np.float32)
    W_ih = np.asarray(W_ih, dtype=np.float32)
    W_hh = np.asarray(W_hh, dtype=np.float32)
    b_ih = np.asarray(b_ih, dtype=np.float32)
    b_hh = np.asarray(b_hh, dtype=np.float32)
    h0 = np.asarray(h0, dtype=np.float32)
    c0 = np.asarray(c0, dtype=np.float32)

    wcomb = _prep_weights(emb, W_ih, W_hh, b_ih, b_hh)

    key = n_steps
    if key not in _cached:
        _cached[key] = build_kernel(n_steps)
    nc = _cached[key]

    in_maps = [
        _prep_core_inputs(x, wcomb, h0, c0, core, n_steps)
        for core in range(NCORES)
    ]
    res = run_bass_kernel_spmd(nc, in_maps, core_ids=list(range(NCORES)))
    kernel.last_results = res

    out = np.empty((B, H), dtype=np.float32)
    for core in range(NCORES):
        ot = res.results[core]["out"]                    # [128, KC*N]
        out[core * N:(core + 1) * N] = (
            ot.reshape(128, KC, N).transpose(2, 1, 0).reshape(N, H)
        )
    return out


# revision 12
# speedup vs baseline: 1.2153x; 1.2153x over previous
"""Trainium2 Bass kernel for a char-LSTM (nn_CharsLstm) — v5: x-path off PE,
fp16 elementwise, stall-free FIFO schedule.

Reference computation (B=4096 words, T=30 chars, D=512 emb, H=1024 hidden,
V=128 chars):
    xe = emb[x]                        # [B, T, D]
    scan over t: gates = xt @ W_ih.T + b_ih + h @ W_hh.T + b_hh
                 i, f, g, o = split(gates, 4)
                 c = sig(f)*c + sig(i)*tanh(g); h = sig(o)*tanh(c)
    return h                           # [B, H]

Strategy:
  - Data parallel: batch 4096 -> 8 cores x 512 words. No collectives.
  - x-path OFF the PE: xw[b,t] = Wc[:, x[b,t]] (Wc = W_ih@emb.T + biases,
    gate rows permuted [i,f,o,g], x16, g x32) is host-gathered per step to
    fp16 [128, KC, 4, 512], streamed to SBUF double-buffered, and added to
    the h-path PSUM gates by the DVE (the only non-PE engine with PSUM
    access; GPSIMD's PSUM port is physically absent, DMA has no route).
    PE: 4 fp8 DoubleRow pairs per tile (K=1024), 128 MMs/step vs v1's 160.
  - Probed per-op costs (N=1024/partition): DVE f32psum+f16 add 1095ns,
    DVE f16 TT 600ns (2x mode; fp8-out falls to 1271), DVE f16
    tensor_scalar 112ns (4x), ACT (N+352)/1.2, GPSIMD ~2.2ns/elem flat.
    So all state math is fp16 (c-state too: 2^-11 rounding over 30 steps
    adds ~1e-3 rel err) and GPSIMD only gets self-contained ops.
  - FIFO discipline (v2/v4 measured 42-44us/step because in-order engine
    FIFOs waited on cross-engine producers): every op is emitted only at a
    point where its inputs are >= 2 chunks old. Ladder per chunk j:
      j   : MMs -> DVE add_if/add_og -> ACT sigmoid (N=2048, 1/16 descale)
            -> m(j) = c*sig_f on GPS (waits sig, GPS has slack)
      j+2 : DVE t_g(j) = 2*sig(2g)-1; u(j) = sig_i*t_g on GPS (0-3) or
            DVE (4-7)
      j+3 : DVE c(j) = m + u   (GPS producers ~3us old -> no wait)
      pair done: ACT tanh pair, h = sig_o*tanh(c): GPS pairs (0,1),(2,3),
            (4,5); chunks 6,7 singles on DVE at the step tail
  - Step boundary: front-load a0-a2 of chunks 0/1 (24 MMs, reads h 0-5)
    covers the tail chain; chunks 6/7 use SPLIT sigmoids (if-half early,
    og-half N=1024) and all-DVE immediate post ops so h6/h7 (which gate
    the a3 pairs) arrive ~4.3us after the last matmul.
  - Engine budget/step: PE 23.3us, DVE ~23.5, ACT ~21.4, GPSIMD ~18.
"""

import numpy as np
import ml_dtypes

import concourse.bacc as bacc
import concourse.mybir as mybir
import concourse.tile as tile
from concourse.bass_utils import run_bass_kernel_spmd

B, T, D, H, V = 4096, 30, 512, 1024, 128
NCORES = 8
N = B // NCORES          # batch per core (matmul moving free dim)
KC = H // 128            # 8 h-chunks of 128 rows
NPAIR = KC // 2          # 4 DoubleRow h-pairs (x-path is off the PE)
F32 = mybir.dt.float32
F16 = mybir.dt.float16
BF16 = mybir.dt.bfloat16
FP8 = mybir.dt.float8e4
FP8NP = ml_dtypes.float8_e4m3
SIG = mybir.ActivationFunctionType.Sigmoid
TANH = mybir.ActivationFunctionType.Tanh
DR = mybir.MatmulPerfMode.DoubleRow
GSCALE = 16.0            # weights carry x16; ACT descales gates by 1/16
XW_STEP = KC * 4 * N     # fp16 elems per partition per step (16384)

_cached = {}


def build_kernel(n_steps=T, repeat=1, xw_static=False):
    nc = bacc.Bacc("TRN2", target_bir_lowering=False)

    whh_d = nc.dram_tensor("whh", [128, NPAIR * 2 * 4096], FP8,
                           kind="ExternalInput")
    xw_d = nc.dram_tensor("xw", [128, n_steps * XW_STEP], F16,
                          kind="ExternalInput")
    h0_d = nc.dram_tensor("h0t", [128, KC * N], FP8, kind="ExternalInput")
    c0_d = nc.dram_tensor("c0t", [128, KC * N], F16, kind="ExternalInput")
    out_d = nc.dram_tensor("out", [128, KC * N], F32, kind="ExternalOutput")

    with tile.TileContext(nc) as tc:
        with (
            tc.tile_pool(name="weights", bufs=1) as wpool,
            tc.tile_pool(name="state", bufs=2) as spool,
            tc.tile_pool(name="xws", bufs=2) as xpool,
            tc.tile_pool(name="stage", bufs=2) as gpool,
            tc.tile_pool(name="sigs", bufs=6) as s4pool,
            tc.tile_pool(name="tmps", bufs=4) as tpool,
            tc.tile_pool(name="psum_if", bufs=2, space="PSUM") as pool_if,
            tc.tile_pool(name="psum_og", bufs=2, space="PSUM") as pool_og,
        ):
            # DMA emission order = consumption order.
            wcomb = wpool.tile([128, NPAIR, 2, 4096], FP8, tag="wcomb")
            nc.sync.dma_start(out=wcomb[:, 0, :, :], in_=whh_d[:, 0:8192])
            ht = spool.tile([128, KC, N], FP8, tag="ht")
            nc.sync.dma_start(out=ht[:, :, :], in_=h0_d[:, :])
            for a in range(1, NPAIR):
                nc.sync.dma_start(out=wcomb[:, a, :, :],
                                  in_=whh_d[:, a * 8192:(a + 1) * 8192])
            # c-state in chunk-pair tiles (tanh batches two chunks)
            ctp = {}
            for j0 in (0, 2, 4, 6):
                tile_c = wpool.tile([128, 2, 512], F16, tag=f"ct{j0}",
                                    name=f"ct{j0}")
                nc.sync.dma_start(out=tile_c,
                                  in_=c0_d[:, j0 * N:(j0 + 2) * N])
                ctp[j0] = tile_c

            def ct(j):
                return ctp[j - j % 2][:, j % 2, :]

            xw_st = None
            if xw_static:
                xw_st = wpool.tile([128, KC, 4, N], F16, tag="xw_st")
                nc.sync.dma_start(out=xw_st, in_=xw_d[:, 0:XW_STEP])

            total = n_steps * repeat
            state = {}   # per-chunk tiles of the CURRENT step: s4, m, u

            for s in range(total):
                t = s % n_steps
                last = s == total - 1
                ht_next = None if last else spool.tile([128, KC, N], FP8,
                                                       tag="ht")

                if xw_static:
                    xw = xw_st
                else:
                    xw = xpool.tile([128, KC, 4, N], F16, tag="xw",
                                    name=f"xw_{s}")
                    nc.sync.dma_start(
                        out=xw,
                        in_=xw_d[:, t * XW_STEP:(t + 1) * XW_STEP])

                def emit_mms(pt, j, qs, a_list):
                    for a in a_list:
                        for qi, q in enumerate(qs):
                            m0 = q * H + j * 128
                            nc.tensor.matmul(
                                pt[:, qi, :],
                                wcomb[:, a, :, m0:m0 + 128],
                                ht[:, 2 * a:2 * a + 2, :],
                                start=(a == 0), stop=(a == NPAIR - 1),
                                perf_mode=DR,
                            )

                def emit_stage1(j, front=None):
                    # matmuls -> DVE xw adds -> sigmoid -> m (early)
                    if front is None:
                        ptif = pool_if.tile([128, 2, 512], F32, tag="pif",
                                            name=f"pif_{s}_{j}")
                        ptog = pool_og.tile([128, 2, 512], F32, tag="pog",
                                            name=f"pog_{s}_{j}")
                        emit_mms(ptif, j, (0, 1), range(NPAIR))
                    else:
                        ptif, ptog = front
                        emit_mms(ptif, j, (0, 1), [NPAIR - 1])
                    spre = gpool.tile([128, 4, 512], F32, tag="spre",
                                      name=f"spre_{s}_{j}")
                    nc.vector.tensor_add(spre[:, 0:2, :], ptif,
                                         xw[:, j, 0:2, :])
                    s4 = s4pool.tile([128, 4, 512], F16, tag="s4",
                                     name=f"s4_{s}_{j}")
                    if j >= 6:
                        # split sigmoid: if-half fires while og matmuls run,
                        # shortening the step-boundary serial chain
                        nc.scalar.activation(out=s4[:, 0:2, :],
                                             in_=spre[:, 0:2, :],
                                             func=SIG, scale=1.0 / GSCALE)
                    if front is None:
                        emit_mms(ptog, j, (2, 3), range(NPAIR))
                    else:
                        emit_mms(ptog, j, (2, 3), [NPAIR - 1])
                    nc.vector.tensor_add(spre[:, 2:4, :], ptog,
                                         xw[:, j, 2:4, :])
                    if j >= 6:
                        nc.scalar.activation(out=s4[:, 2:4, :],
                                             in_=spre[:, 2:4, :],
                                             func=SIG, scale=1.0 / GSCALE)
                    else:
                        nc.scalar.activation(out=s4, in_=spre, func=SIG,
                                             scale=1.0 / GSCALE)
                    # m = c_old * sig_f: GPS for 0-5 (slack-timed; consumed
                    # 3 chunks later), DVE for the tail chunks
                    m = tpool.tile([128, 512], F16, tag="m",
                                   name=f"m_{s}_{j}")
                    eng = nc.vector if j >= 6 else nc.gpsimd
                    eng.tensor_mul(m, ct(j), s4[:, 1, :])
                    state[j] = {"s4": s4, "m": m}

                def emit_tgu(j):
                    # t_g = 2*sig(2g)-1 (DVE, 4x mode); u = sig_i * t_g
                    st = state[j]
                    tg = tpool.tile([128, 512], F16, tag="tg",
                                    name=f"tg_{s}_{j}")
                    nc.vector.tensor_scalar(tg, st["s4"][:, 3, :], 2.0, -1.0,
                                            mybir.AluOpType.mult,
                                            mybir.AluOpType.add)
                    u = tpool.tile([128, 512], F16, tag="u",
                                   name=f"u_{s}_{j}")
                    eng = nc.gpsimd if j <= 3 else nc.vector
                    eng.tensor_mul(u, st["s4"][:, 0, :], tg)
                    st["u"] = u

                def emit_cts(j):
                    st = state[j]
                    nc.vector.tensor_add(ct(j), st["m"], st["u"])

                def emit_stage2_pair(j0, h_dst, lst):
                    # tanh over the c pair, then h = sig_o * tanh(c)
                    t_c = tpool.tile([128, 2, 512], F16, tag="t_c",
                                     name=f"tc_{s}_{j0}")
                    nc.scalar.activation(out=t_c, in_=ctp[j0], func=TANH)
                    for k, j in enumerate((j0, j0 + 1)):
                        s_o = state[j]["s4"][:, 2, :]
                        if lst:
                            ho = tpool.tile([128, 512], F32, tag="hout",
                                            name=f"ho_{s}_{j}")
                            nc.vector.tensor_mul(ho, s_o, t_c[:, k, :])
                            nc.sync.dma_start(
                                out=out_d[:, j * N:(j + 1) * N], in_=ho)
                        else:
                            nc.gpsimd.tensor_mul(h_dst[:, j, :], s_o,
                                                 t_c[:, k, :])

                def emit_stage2_single(j, h_dst, lst):
                    t_c = tpool.tile([128, 512], F16, tag="t_c1",
                                     name=f"tc1_{s}_{j}")
                    nc.scalar.activation(out=t_c, in_=ct(j), func=TANH)
                    s_o = state[j]["s4"][:, 2, :]
                    if lst:
                        ho = tpool.tile([128, 512], F32, tag="hout1",
                                        name=f"ho1_{s}_{j}")
                        nc.vector.tensor_mul(ho, s_o, t_c)
                        nc.sync.dma_start(out=out_d[:, j * N:(j + 1) * N],
                                          in_=ho)
                    else:
                        nc.vector.tensor_mul(h_dst[:, j, :], s_o, t_c)

                # ---- step body ----
                # front-load a0-a2 of chunks 0/1 (reads h chunks 0-5 only;
                # h chunks 6/7 of the previous step are still in flight)
                fr = {}
                for j in (0, 1):
                    fr[j] = (pool_if.tile([128, 2, 512], F32, tag="pif",
                                          name=f"pif_{s}_{j}"),
                             pool_og.tile([128, 2, 512], F32, tag="pog",
                                          name=f"pog_{s}_{j}"))
                    emit_mms(fr[j][0], j, (0, 1), range(NPAIR - 1))
                    emit_mms(fr[j][1], j, (2, 3), range(NPAIR - 1))

                state = {}
                emit_stage1(0, front=fr[0])
                emit_stage1(1, front=fr[1])
                for j in range(2, 7):
                    emit_stage1(j)
                    emit_tgu(j - 2)
                    if j - 3 >= 0:
                        emit_cts(j - 3)
                    if j - 3 == 1:
                        emit_stage2_pair(0, ht_next, last)
                    elif j - 3 == 3:
                        emit_stage2_pair(2, ht_next, last)
                # (4,5) finishes BEFORE chunk 7's sigmoids hit the ACT queue
                # so its h-muls (read by next step's a2 front-load) are on
                # time; chunk 7 then runs the short all-DVE tail for h6/h7
                emit_tgu(5)
                emit_cts(4)
                emit_cts(5)
                emit_stage2_pair(4, ht_next, last)
                emit_stage1(7)
                emit_tgu(6)
                emit_cts(6)
                emit_stage2_single(6, ht_next, last)
                emit_tgu(7)
                emit_cts(7)
                emit_stage2_single(7, ht_next, last)
                ht = ht_next

    nc.compile()
    return nc


def _prep_core_inputs(x, wcomb, h0, c0, core, n_steps=T):
    whh, xtab = wcomb
    sl = slice(core * N, (core + 1) * N)
    x_c = np.asarray(x[sl]).astype(np.int64)     # [N, T]
    # int16 view: np.take on float16 falls off numpy's fast path (~17x)
    xv = xtab.view(np.int16)
    xw_all = np.empty((128, n_steps, KC, 4, N), dtype=np.int16)
    for t in range(n_steps):
        xw_all[:, t] = np.take(xv, x_c[:, t], axis=3)
    xw_all = xw_all.reshape(128, n_steps * XW_STEP).view(np.float16)
    h0t = np.ascontiguousarray(
        h0[sl].reshape(N, KC, 128).transpose(2, 1, 0).reshape(128, KC * N)
    ).astype(FP8NP)
    c0t = np.ascontiguousarray(
        c0[sl].reshape(N, KC, 128).transpose(2, 1, 0).reshape(128, KC * N)
    ).astype(np.float16)
    return {"whh": whh, "xw": xw_all, "h0t": h0t, "c0t": c0t}


def _prep_weights(emb, W_ih, W_hh, b_ih, b_hh):
    # gate reorder [i, f, o, g]
    perm = np.concatenate([np.arange(0, H), np.arange(H, 2 * H),
                           np.arange(3 * H, 4 * H), np.arange(2 * H, 3 * H)])
    # g rows (block 3 after reorder) carry an extra x2: the single sigmoid
    # yields sigmoid(2g) there and tanh(g) = 2*sig(2g) - 1
    gate_scale = np.repeat([1.0, 1.0, 1.0, 2.0], H)[:, None] * GSCALE
    whh_s = gate_scale * W_hh[perm]                      # [4H, H] x16 (g x32)

    whh = np.zeros((128, NPAIR, 2, 4 * H), dtype=np.float32)
    for a in range(NPAIR):
        for r in range(2):
            k = 2 * a + r
            whh[:, a, r, :] = whh_s[:, k * 128:(k + 1) * 128].T
    whh = np.ascontiguousarray(
        whh.reshape(128, NPAIR * 2 * 4096)).astype(FP8NP)

    # x-path table for the host gather: xtab[p, j, q, v]
    wc = W_ih @ emb.T + (b_ih + b_hh)[:, None]           # [4H, V]
    wc = gate_scale * wc[perm]                           # x16 (g x32)
    xtab = np.ascontiguousarray(
        wc.reshape(4, KC, 128, V).transpose(2, 1, 0, 3)).astype(np.float16)
    return whh, xtab


def kernel(x, emb, W_ih, W_hh, b_ih, b_hh, h0, c0, n_steps=T):
    x = np.asarray(x)
    emb = np.asarray(emb, dtype=np.float32)
    W_ih = np.asarray(W_ih, dtype=np.float32)
    W_hh = np.asarray(W_hh, dtype=np.float32)
    b_ih = np.asarray(b_ih, dtype=np.float32)
    b_hh = np.asarray(b_hh, dtype=np.float32)
    h0 = np.asarray(h0, dtype=np.float32)
    c0 = np.asarray(c0, dtype=np.float32)

    wcomb = _prep_weights(emb, W_ih, W_hh, b_ih, b_hh)

    key = n_steps
    if key not in _cached:
        _cached[key] = build_kernel(n_steps)
    nc = _cached[key]

    in_maps = [
        _prep_core_inputs(x, wcomb, h0, c0, core, n_steps)
        for core in range(NCORES)
    ]
    res = run_bass_kernel_spmd(nc, in_maps, core_ids=list(range(NCORES)))
    kernel.last_results = res

    out = np.empty((B, H), dtype=np.float32)
    for core in range(NCORES):
        ot = res.results[core]["out"]                    # [128, KC*N]
        out[core * N:(core + 1) * N] = (
            ot.reshape(128, KC, N).transpose(2, 1, 0).reshape(N, H)
        )
    return out
